# revision 28
# baseline (speedup 1.0000x reference)
"""Trainium2 Bass kernel for nn_MultiHeadAttention_65987877535893.

MHA with RoPE: B=2, S=2048, D=1024, H=16, Dh=64, causal mask.

Sharding (8 cores): data-parallel over B (x2) x tensor-parallel over heads
(x4 -> 4 heads/core).  Each core computes, for its batch b and head group g:
  QKV projections (column-sharded W), RoPE, causal attention, and a partial
  output  A_g @ Wo_g  (row-sharded Wo).  Host sums the 4 partials per batch.

v2.2 (bf16, software-pipelined): all matmul inputs bf16 (PSUM accum fp32).
 - Wq/Wk columns host-permuted to pair-interleaved order per head (x1_i at
   partition 2i, x2_i at 2i+1), so RoPE is 4 full-width [128,512] DVE ops:
   r1 = p*cc, r2 = p*ss (cross-term sign baked into ss), r2s = pairswap(r2)
   via stream_shuffle, q = r1 + r2s.
 - Causal masking of diagonal blocks via gpsimd affine_select
   (iota = col - kappa >= 0) on the exp'd probs -- no mask tensor.
 - Scores emitted one kb ahead of PV; projection work for slice m+1 is
   emitted in chunks interleaved into macro m's attention stream so the PE
   never drains (keeps HAM at full clock); p=0 normalize deferred past the
   next group's first scores.
 - PSUM: pp(proj/sbc/outproj) 2 banks + st 4 banks + at 2 banks = 8.
"""

import os
import sys

sys.path.insert(0, "/opt/trn_rl_repo")
os.environ.setdefault("MYCRO_LOCAL_CACHE", "1")

import numpy as np

import concourse.bass as bass
import concourse.bacc as bacc
import concourse.mybir as mybir
import concourse.tile as tile
from concourse.bass_utils import run_bass_kernel_spmd

F32 = mybir.dt.float32
BF16 = mybir.dt.bfloat16

B, S, D = 2, 2048, 1024
H = 16
DH = 64
HPC = 4           # heads per core
DG = HPC * DH     # 256
N_CORES = 8
KO = D // 128     # 8 contraction chunks
N_SLICES = S // 512   # 4 q/s slices
EXP_SCALE = float(DH) ** -0.5  # 0.125
Exp = mybir.ActivationFunctionType.Exp
GE = mybir.AluOpType.is_ge

# pair-swap within quadrants: 0<->1, 2<->3, ...
SWAP_MASK = [i ^ 1 for i in range(32)]


def build_nc():
    nc = bacc.Bacc()

    # weights are host-prearranged to [128, ko*m] so DMA runs are contiguous
    xT = nc.dram_tensor("xT", [D, S], BF16, kind="ExternalInput")
    wqa = nc.dram_tensor("wqa", [128, KO * 128], BF16, kind="ExternalInput")
    wqb = nc.dram_tensor("wqb", [128, KO * 128], BF16, kind="ExternalInput")
    wka = nc.dram_tensor("wka", [128, KO * 128], BF16, kind="ExternalInput")
    wkb = nc.dram_tensor("wkb", [128, KO * 128], BF16, kind="ExternalInput")
    wv = nc.dram_tensor("wv", [128, KO * DG], BF16, kind="ExternalInput")
    wo = nc.dram_tensor("wo", [128, 2 * D], BF16, kind="ExternalInput")
    ccT = nc.dram_tensor("ccT", [128, S], F32, kind="ExternalInput")
    ssT = nc.dram_tensor("ssT", [128, S], F32, kind="ExternalInput")
    o_part = nc.dram_tensor("o_part", [S, D], BF16, kind="ExternalOutput")

    with tile.TileContext(nc) as tc:
        import contextlib

        ctx = contextlib.ExitStack()
        with ctx:
            persist = ctx.enter_context(tc.tile_pool(name="persist", bufs=1))
            work = ctx.enter_context(tc.tile_pool(name="work", bufs=2))

            # ---- persistent SBUF tensors ----
            qr = [[persist.tile([128, 512], BF16, tag=f"qr{p}_{ms}", name=f"qr{p}_{ms}")
                   for ms in range(N_SLICES)] for p in range(2)]
            kr = [[persist.tile([128, 512], BF16, tag=f"kr{p}_{ms}", name=f"kr{p}_{ms}")
                   for ms in range(N_SLICES)] for p in range(2)]
            vt = [persist.tile([128, 4, HPC, DH + 1], BF16, tag=f"vt{ms}", name=f"vt{ms}")
                  for ms in range(N_SLICES)]
            atn = [[persist.tile([128, 512], BF16, tag=f"atn{p}_{ms}", name=f"atn{p}_{ms}")
                    for ms in range(N_SLICES)] for p in range(2)]
            cc_sb = persist.tile([128, S], F32, tag="cc", name="cc")
            ss_sb = persist.tile([128, S], F32, tag="ss", name="ss")
            wqa_sb = persist.tile([128, KO, 128], BF16, tag="wqa", name="wqa")
            wqb_sb = persist.tile([128, KO, 128], BF16, tag="wqb", name="wqb")
            wka_sb = persist.tile([128, KO, 128], BF16, tag="wka", name="wka")
            wkb_sb = persist.tile([128, KO, 128], BF16, tag="wkb", name="wkb")
            wv_sb = persist.tile([128, KO, DG], BF16, tag="wv", name="wv")
            wo_sb = persist.tile([128, 2, D], BF16, tag="wo", name="wo")

            # xts for slice 0 first so the first proj can start ASAP
            xTr = xT.rearrange("(ko p) s -> p ko s", p=128)
            xts0 = work.tile([128, KO, 512], BF16, tag="xts", name="xts0")
            nc.sync.dma_start(xts0[:, 0:4], xTr[:, 0:4, 0:512])
            nc.sync.dma_start(xts0[:, 4:8], xTr[:, 4:8, 0:512])

            # upfront DMAs: only what macro-0 p=0 needs, to keep the early
            # HBM window small (xts0 0.5M + wqa/wka 0.5M + wv 0.5M + cc/ss
            # first halves 1M); the rest is issued later in the stream
            nc.scalar.dma_start(
                wqa_sb[:], wqa.rearrange("p (ko m) -> p ko m", m=128))
            nc.scalar.dma_start(
                wka_sb[:], wka.rearrange("p (ko m) -> p ko m", m=128))
            nc.scalar.dma_start(
                wv_sb[:], wv.rearrange("p (ko m) -> p ko m", m=DG))
            nc.gpsimd.dma_start(cc_sb[:, 0:1024], ccT[:, 0:1024])
            nc.gpsimd.dma_start(ss_sb[:, 0:1024], ssT[:, 0:1024])

            # deferred input DMAs ride the scalar queue: descriptor-gen there
            # sits behind pending exp instructions, so the transfers really do
            # start late instead of racing ahead and starving the early loads
            def c_dma_w1():
                nc.scalar.dma_start(
                    wqb_sb[:], wqb.rearrange("p (ko m) -> p ko m", m=128))
                nc.scalar.dma_start(
                    wkb_sb[:], wkb.rearrange("p (ko m) -> p ko m", m=128))

            def c_dma_w2():
                nc.scalar.dma_start(cc_sb[:, 1024:2048], ccT[:, 1024:2048])
                nc.scalar.dma_start(ss_sb[:, 1024:2048], ssT[:, 1024:2048])
                nc.scalar.dma_start(
                    wo_sb[:], wo.rearrange("p (ko m) -> p ko m", m=D))

            onesf = persist.tile([128, 16], F32, tag="onesf", name="onesf")
            ones1 = persist.tile([1, 64], BF16, tag="ones1", name="ones1")
            nc.vector.memset(onesf[:], 1.0)
            nc.vector.memset(ones1[:], 1.0)
            for ms in range(N_SLICES):
                nc.vector.tensor_copy(
                    vt[ms][:, :, :, DH],
                    onesf[:, 0:16].rearrange("p (a b) -> p a b", b=HPC))

            # PSUM pools: pp 2 banks + st 4 banks + at 2 banks = 8
            pp = ctx.enter_context(tc.tile_pool(name="pp", bufs=2, space="PSUM"))
            stp = ctx.enter_context(tc.tile_pool(name="stp", bufs=2, space="PSUM"))
            atp = ctx.enter_context(tc.tile_pool(name="atp", bufs=1, space="PSUM"))

            # ---------------- projection chunk emitters ----------------
            xts_cell = {0: xts0}

            def c_dma(m):
                def f():
                    xts = work.tile([128, KO, 512], BF16, tag="xts",
                                    name=f"xts{m}")
                    xts_cell[m] = xts
                    sl = slice(512 * m, 512 * (m + 1))
                    nc.sync.dma_start(xts[:, 0:4], xTr[:, 0:4, sl])
                    nc.sync.dma_start(xts[:, 4:8], xTr[:, 4:8, sl])
                return f

            def _pp_tile(name):
                return pp.tile([128, 512], F32, tag="pp", name=name)

            def _st_tile(name):
                # startup only: borrow the (idle) score pool's banks so the
                # first projection chunks pipeline 4-deep instead of 2-deep
                return stp.tile([128, 2, 512], F32, tag="st", name=name)[:, 0, :]

            def c_qk(m, w_sb, dst, mk=_pp_tile):
                def f():
                    xts = xts_cell[m]
                    sl = slice(512 * m, 512 * (m + 1))
                    pj = mk("pj")
                    for ko in range(KO):
                        nc.tensor.matmul(pj[:], w_sb[:, ko], xts[:, ko],
                                         start=(ko == 0), stop=(ko == KO - 1))
                    r1 = work.tile([128, 512], BF16, tag="r1", name="r1")
                    r2 = work.tile([128, 512], BF16, tag="r2", name="r2")
                    r2s = work.tile([128, 512], BF16, tag="r2s", name="r2s")
                    nc.vector.tensor_mul(r1[:], pj[:], cc_sb[:, sl])
                    nc.vector.tensor_mul(r2[:], pj[:], ss_sb[:, sl])
                    nc.vector.stream_shuffle(r2s[:], r2[:], SWAP_MASK)
                    nc.vector.tensor_add(dst[m][:], r1[:], r2s[:])
                return f

            def c_v(m, half, mk=_pp_tile):
                def f():
                    xts = xts_cell[m]
                    pv = mk("pv")
                    for sc in range(2):
                        xsl = slice(128 * (2 * half + sc),
                                    128 * (2 * half + sc) + 128)
                        for ko in range(KO):
                            nc.tensor.matmul(
                                pv[:, 256 * sc:256 * sc + 256],
                                xts[:, ko, xsl], wv_sb[:, ko],
                                start=(ko == 0), stop=(ko == KO - 1))
                    nc.vector.tensor_copy(
                        vt[m][:, 2 * half:2 * half + 2, :, 0:DH],
                        pv[:].rearrange("p (sc h d) -> p sc h d", sc=2, d=DH))
                return f

            def proj_chunks(m):
                return [c_dma(m),
                        c_qk(m, wqa_sb, qr[0]), c_qk(m, wka_sb, kr[0]),
                        c_v(m, 0), c_v(m, 1),
                        c_qk(m, wqb_sb, qr[1]), c_qk(m, wkb_sb, kr[1])]

            # slice 0 (xts0 already DMA'd above): emit what macro-0 p=0
            # needs up front, pipelined through the still-idle score pool;
            # QB/KB interleave into the p=0 stream
            c_qk(0, wqa_sb, qr[0])()
            c_qk(0, wka_sb, kr[0])()
            c_v(0, 0)()
            c_v(0, 1)()
            pending = [c_dma_w1] + proj_chunks(0)[5:]
            need_before_p1 = len(pending)   # wqb/wkb+QB0/KB0 precede p=1

            # ---------------- attention ----------------
            def normalize(p, m, at):
                def f():
                    for a in range(2):
                        ssb = work.tile([1, 512], BF16, tag="ssb", name="ssb")
                        rbc = work.tile([64, 512], F32, tag="rbc", name="rbc")
                        nc.vector.tensor_copy(ssb[:], at[a][DH:DH + 1, :])
                        sbc = pp.tile([128, 512], F32, tag="pp", name="sbc")
                        nc.tensor.matmul(sbc[0:64, :], ones1, ssb[:],
                                         start=True, stop=True)
                        nc.vector.reciprocal_approx_fast(rbc[:], sbc[0:64, :])
                        nc.vector.tensor_mul(
                            atn[p][m][64 * a:64 * a + 64, :], at[a][0:DH, :],
                            rbc[:])
                return f

            def outproj_chunk(m, sc):
                def f():
                    scl = slice(128 * (sc % 4), 128 * (sc % 4) + 128)
                    osb = work.tile([128, D], BF16, tag="osb", name="osb")
                    po = [pp.tile([128, 512], F32, tag="pp", name=f"po{nh}")
                          for nh in range(2)]
                    for nh in range(2):
                        for ksub in range(2):
                            nc.tensor.matmul(
                                po[nh][:], atn[ksub][m][:, scl],
                                wo_sb[:, ksub, 512 * nh:512 * nh + 512],
                                start=(ksub == 0), stop=(ksub == 1))
                    # split the PSUM->SBUF copies across engines
                    nc.vector.tensor_copy(osb[:, 0:512], po[0][:])
                    nc.scalar.copy(osb[:, 512:1024], po[1][:])
                    nc.sync.dma_start(o_part[128 * sc:128 * sc + 128, :], osb[:])
                return f

            deferred = [None]   # normalize of the previous (m, p) group

            for m in range(N_SLICES):
                if m > 0:
                    pending = [outproj_chunk(m - 1, sc)
                               for sc in range(4 * (m - 1), 4 * (m - 1) + 4)]
                    if m == 1:
                        pending.insert(0, c_dma_w2)
                    need_before_p1 = 0
                if m + 1 < N_SLICES:
                    pending.extend(proj_chunks(m + 1))
                nkb = 4 * m + 4
                iters = 2 * nkb
                it = 0
                emitted = 0
                for p in range(2):
                    sts = {}

                    def emit_scores(kb, p=p, m=m, sts=sts):
                        st = stp.tile([128, 2, 512], F32, tag="st", name="st")
                        sts[kb] = st
                        ksl = slice(128 * (kb % 4), 128 * (kb % 4) + 128)
                        c0 = 128 * (kb % 4) if (kb // 4 == m) else 0
                        for a in range(2):
                            nc.tensor.matmul(
                                st[:, a, c0:],
                                kr[p][kb // 4][64 * a:64 * a + 64, ksl],
                                qr[p][m][64 * a:64 * a + 64, c0:],
                                start=True, stop=True)

                    if p == 1:
                        while emitted < need_before_p1:
                            pending[emitted]()
                            emitted += 1
                    emit_scores(0)
                    # flush the previous group's normalize now, before the
                    # at-pool slots are reallocated below (WAR ordering)
                    if deferred[0] is not None:
                        deferred[0]()
                        deferred[0] = None
                    at = [atp.tile([DH + 1, 512], F32, tag=f"at{a}",
                                   name=f"at{a}") for a in range(2)]
                    for kb in range(nkb):
                        diag = (kb // 4 == m)
                        c0 = 128 * (kb % 4) if diag else 0
                        st = sts.pop(kb)
                        pt = work.tile([128, 2, 512], BF16, tag="pt", name="pt",
                                       bufs=4)
                        if diag:
                            # exp the 128-col triangle region first so the
                            # causal mask (gpsimd) runs while the rest of the
                            # row exps concurrently -- keeps the mask off the
                            # scores->PV critical path
                            c1 = c0 + 128
                            nc.scalar.activation(
                                pt[:, :, c0:c1], st[:, :, c0:c1], Exp,
                                scale=EXP_SCALE)
                            nc.gpsimd.affine_select(
                                pt[:, :, c0:c1], pt[:, :, c0:c1],
                                pattern=[[0, 2], [1, 128]],
                                compare_op=GE, fill=0.0,
                                base=0, channel_multiplier=-1)
                            if c1 < 512:
                                nc.scalar.activation(
                                    pt[:, :, c1:], st[:, :, c1:], Exp,
                                    scale=EXP_SCALE)
                        else:
                            nc.scalar.activation(
                                pt[:, :, c0:], st[:, :, c0:], Exp,
                                scale=EXP_SCALE)
                        if kb + 1 < nkb:
                            emit_scores(kb + 1)
                        for a in range(2):
                            nc.tensor.matmul(
                                at[a][:, c0:], vt[kb // 4][:, kb % 4, 2 * p + a],
                                pt[:, a, c0:],
                                start=(kb == 0), stop=(kb == nkb - 1))
                        # interleave pending chunks (outproj m-1, proj m+1)
                        it += 1
                        want = (it * len(pending)) // max(iters, 1)
                        while emitted < want and emitted < len(pending):
                            pending[emitted]()
                            emitted += 1

                    deferred[0] = normalize(p, m, at)
                while emitted < len(pending):
                    pending[emitted]()
                    emitted += 1
                pending = []

            # tail: last normalize + last macro's output projection
            deferred[0]()
            for sc in range(4 * (N_SLICES - 1), 4 * N_SLICES):
                outproj_chunk(N_SLICES - 1, sc)()

    nc.finalize()
    return nc


def _to_bf16(x):
    from ml_dtypes import bfloat16
    return np.asarray(x, dtype=np.float32).astype(bfloat16)


def prep_inputs(hidden_states, cos, sin, attention_mask, Wq, Wk, Wv, Wo):
    """Host-side sharding/layout prep. Returns in_maps for the 8 cores."""
    hs = np.asarray(hidden_states, dtype=np.float32)
    cos = np.asarray(cos, dtype=np.float32)
    sin = np.asarray(sin, dtype=np.float32)
    Wq = np.asarray(Wq, dtype=np.float32)
    Wk = np.asarray(Wk, dtype=np.float32)
    Wv = np.asarray(Wv, dtype=np.float32)
    Wo = np.asarray(Wo, dtype=np.float32)

    # rope tables in pair-interleaved layout, 2 heads (128 partitions) per tile
    idx = np.empty(64, dtype=np.int64)
    idx[0::2] = np.arange(32)
    idx[1::2] = np.arange(32)
    cc1 = cos.T[idx]                      # [64, S]
    ss1 = sin.T[idx].copy()               # [64, S]
    ss1[1::2] *= -1.0
    ccT = np.ascontiguousarray(np.tile(cc1, (2, 1)), dtype=np.float32)  # [128,S]
    ssT = np.ascontiguousarray(np.tile(ss1, (2, 1)), dtype=np.float32)

    # per-head column permutation of Wq/Wk into interleaved order
    perm = np.empty(64, dtype=np.int64)
    perm[0::2] = np.arange(32)        # x1_i = dim i
    perm[1::2] = np.arange(32) + 32   # x2_i = dim 32+i

    xTs = [np.ascontiguousarray(_to_bf16(hs[b].T)) for b in range(B)]

    def _prearr(w):
        # [ko*128, m] -> [128, ko*m] so per-partition DMA runs are contiguous
        ko, m = w.shape[0] // 128, w.shape[1]
        return np.ascontiguousarray(_to_bf16(
            w.reshape(ko, 128, m).transpose(1, 0, 2).reshape(128, ko * m)))

    in_maps = []
    for c in range(N_CORES):
        b, g = c // 4, c % 4
        hsl = slice(DG * g, DG * (g + 1))
        wq_g = Wq[:, hsl].reshape(D, HPC, DH)[:, :, perm]   # [D, 4, 64]
        wk_g = Wk[:, hsl].reshape(D, HPC, DH)[:, :, perm]
        in_maps.append({
            "xT": xTs[b],
            "wqa": _prearr(wq_g[:, 0:2].reshape(D, 128)),
            "wqb": _prearr(wq_g[:, 2:4].reshape(D, 128)),
            "wka": _prearr(wk_g[:, 0:2].reshape(D, 128)),
            "wkb": _prearr(wk_g[:, 2:4].reshape(D, 128)),
            "wv": _prearr(Wv[:, hsl]),
            "wo": _prearr(Wo[hsl, :]),
            "ccT": ccT,
            "ssT": ssT,
        })
    return in_maps


_NC_CACHE = {}


def get_nc():
    if "nc" not in _NC_CACHE:
        _NC_CACHE["nc"] = build_nc()
    return _NC_CACHE["nc"]


def run(inputs, trace=False):
    """Returns (output [B,S,D] fp32, BassKernelResults)."""
    nc = get_nc()
    in_maps = prep_inputs(**inputs)
    res = run_bass_kernel_spmd(nc, in_maps, list(range(N_CORES)), trace=trace)
    out = np.zeros((B, S, D), dtype=np.float32)
    for c in range(N_CORES):
        out[c // 4] += np.asarray(res.results[c]["o_part"], dtype=np.float32)
    return out, res


def kernel(**inputs):
    return run(inputs, trace=False)[0]


# revision 31
# speedup vs baseline: 1.0472x; 1.0472x over previous
"""Trainium2 Bass kernel for nn_MultiHeadAttention_65987877535893.

MHA with RoPE: B=2, S=2048, D=1024, H=16, Dh=64, causal mask.

Sharding (8 cores): data-parallel over B (x2) x tensor-parallel over heads
(x4 -> 4 heads/core).  Each core computes, for its batch b and head group g:
  QKV projections (column-sharded W), RoPE, causal attention, and a partial
  output  A_g @ Wo_g  (row-sharded Wo).  Host sums the 4 partials per batch.

v2.2 (bf16, software-pipelined): all matmul inputs bf16 (PSUM accum fp32).
 - Wq/Wk columns host-permuted to pair-interleaved order per head (x1_i at
   partition 2i, x2_i at 2i+1), so RoPE is 4 full-width [128,512] DVE ops:
   r1 = p*cc, r2 = p*ss (cross-term sign baked into ss), r2s = pairswap(r2)
   via stream_shuffle, q = r1 + r2s.
 - Causal masking of diagonal blocks via gpsimd affine_select
   (iota = col - kappa >= 0) on the exp'd probs -- no mask tensor.
 - Scores emitted one kb ahead of PV; projection work for slice m+1 is
   emitted in chunks interleaved into macro m's attention stream so the PE
   never drains (keeps HAM at full clock); p=0 normalize deferred past the
   next group's first scores.
 - PSUM: pp(proj/sbc/outproj) 2 banks + st 4 banks + at 2 banks = 8.
"""

import os
import sys

sys.path.insert(0, "/opt/trn_rl_repo")
os.environ.setdefault("MYCRO_LOCAL_CACHE", "1")

import numpy as np

import concourse.bass as bass
import concourse.bacc as bacc
import concourse.mybir as mybir
import concourse.tile as tile
from concourse.bass_utils import run_bass_kernel_spmd

F32 = mybir.dt.float32
BF16 = mybir.dt.bfloat16

B, S, D = 2, 2048, 1024
H = 16
DH = 64
HPC = 4           # heads per core
DG = HPC * DH     # 256
N_CORES = 8
KO = D // 128     # 8 contraction chunks
N_SLICES = S // 512   # 4 q/s slices
EXP_SCALE = float(DH) ** -0.5  # 0.125
Exp = mybir.ActivationFunctionType.Exp
GE = mybir.AluOpType.is_ge

# pair-swap within quadrants: 0<->1, 2<->3, ...
SWAP_MASK = [i ^ 1 for i in range(32)]


def build_nc():
    nc = bacc.Bacc()

    # weights are host-prearranged to [128, ko*m] so DMA runs are contiguous
    xT = nc.dram_tensor("xT", [D, S], BF16, kind="ExternalInput")
    wqa = nc.dram_tensor("wqa", [128, KO * 128], BF16, kind="ExternalInput")
    wqb = nc.dram_tensor("wqb", [128, KO * 128], BF16, kind="ExternalInput")
    wka = nc.dram_tensor("wka", [128, KO * 128], BF16, kind="ExternalInput")
    wkb = nc.dram_tensor("wkb", [128, KO * 128], BF16, kind="ExternalInput")
    wv = nc.dram_tensor("wv", [128, KO * DG], BF16, kind="ExternalInput")
    wo = nc.dram_tensor("wo", [128, 2 * D], BF16, kind="ExternalInput")
    ccT = nc.dram_tensor("ccT", [128, S], F32, kind="ExternalInput")
    ssT = nc.dram_tensor("ssT", [128, S], F32, kind="ExternalInput")
    o_part = nc.dram_tensor("o_part", [S, D], BF16, kind="ExternalOutput")

    with tile.TileContext(nc) as tc:
        import contextlib

        ctx = contextlib.ExitStack()
        with ctx:
            persist = ctx.enter_context(tc.tile_pool(name="persist", bufs=1))
            work = ctx.enter_context(tc.tile_pool(name="work", bufs=2))

            # ---- persistent SBUF tensors ----
            qr = [[persist.tile([128, 512], BF16, tag=f"qr{p}_{ms}", name=f"qr{p}_{ms}")
                   for ms in range(N_SLICES)] for p in range(2)]
            kr = [[persist.tile([128, 512], BF16, tag=f"kr{p}_{ms}", name=f"kr{p}_{ms}")
                   for ms in range(N_SLICES)] for p in range(2)]
            vt = [persist.tile([128, 4, HPC, DH + 1], BF16, tag=f"vt{ms}", name=f"vt{ms}")
                  for ms in range(N_SLICES)]
            atn = [[persist.tile([128, 512], BF16, tag=f"atn{p}_{ms}", name=f"atn{p}_{ms}")
                    for ms in range(N_SLICES)] for p in range(2)]
            cc_sb = persist.tile([128, S], F32, tag="cc", name="cc")
            ss_sb = persist.tile([128, S], F32, tag="ss", name="ss")
            wqa_sb = persist.tile([128, KO, 128], BF16, tag="wqa", name="wqa")
            wqb_sb = persist.tile([128, KO, 128], BF16, tag="wqb", name="wqb")
            wka_sb = persist.tile([128, KO, 128], BF16, tag="wka", name="wka")
            wkb_sb = persist.tile([128, KO, 128], BF16, tag="wkb", name="wkb")
            wv_sb = persist.tile([128, KO, DG], BF16, tag="wv", name="wv")
            wo_sb = persist.tile([128, 2, D], BF16, tag="wo", name="wo")

            # xts for slice 0 first so the first proj can start ASAP
            xTr = xT.rearrange("(ko p) s -> p ko s", p=128)
            xts0 = work.tile([128, KO, 512], BF16, tag="xts", name="xts0")
            nc.sync.dma_start(xts0[:, 0:4], xTr[:, 0:4, 0:512])
            nc.sync.dma_start(xts0[:, 4:8], xTr[:, 4:8, 0:512])

            # upfront DMAs: only what macro-0 p=0 needs, to keep the early
            # HBM window small (xts0 0.5M + wqa/wka 0.5M + wv 0.5M + cc/ss
            # first halves 1M); the rest is issued later in the stream
            nc.scalar.dma_start(
                wqa_sb[:], wqa.rearrange("p (ko m) -> p ko m", m=128))
            nc.scalar.dma_start(
                wka_sb[:], wka.rearrange("p (ko m) -> p ko m", m=128))
            nc.scalar.dma_start(
                wv_sb[:], wv.rearrange("p (ko m) -> p ko m", m=DG))
            nc.gpsimd.dma_start(cc_sb[:, 0:1024], ccT[:, 0:1024])
            nc.gpsimd.dma_start(ss_sb[:, 0:1024], ssT[:, 0:1024])

            def c_dma_w1():
                nc.sync.dma_start(
                    wqb_sb[:], wqb.rearrange("p (ko m) -> p ko m", m=128))
                nc.sync.dma_start(
                    wkb_sb[:], wkb.rearrange("p (ko m) -> p ko m", m=128))

            def c_dma_w2():
                nc.gpsimd.dma_start(cc_sb[:, 1024:2048], ccT[:, 1024:2048])
                nc.gpsimd.dma_start(ss_sb[:, 1024:2048], ssT[:, 1024:2048])
                nc.sync.dma_start(
                    wo_sb[:], wo.rearrange("p (ko m) -> p ko m", m=D))

            onesf = persist.tile([128, 16], F32, tag="onesf", name="onesf")
            ones1 = persist.tile([1, 64], BF16, tag="ones1", name="ones1")
            nc.vector.memset(onesf[:], 1.0)
            nc.vector.memset(ones1[:], 1.0)
            for ms in range(N_SLICES):
                nc.vector.tensor_copy(
                    vt[ms][:, :, :, DH],
                    onesf[:, 0:16].rearrange("p (a b) -> p a b", b=HPC))

            # PSUM pools: pp 2 banks + st 4 banks + at 2 banks = 8
            pp = ctx.enter_context(tc.tile_pool(name="pp", bufs=2, space="PSUM"))
            stp = ctx.enter_context(tc.tile_pool(name="stp", bufs=2, space="PSUM"))
            atp = ctx.enter_context(tc.tile_pool(name="atp", bufs=1, space="PSUM"))

            # ---------------- projection chunk emitters ----------------
            xts_cell = {0: xts0}

            def c_dma(m):
                def f():
                    xts = work.tile([128, KO, 512], BF16, tag="xts",
                                    name=f"xts{m}")
                    xts_cell[m] = xts
                    sl = slice(512 * m, 512 * (m + 1))
                    nc.sync.dma_start(xts[:, 0:4], xTr[:, 0:4, sl])
                    nc.sync.dma_start(xts[:, 4:8], xTr[:, 4:8, sl])
                return f

            def _pp_tile(name):
                return pp.tile([128, 512], F32, tag="pp", name=name)

            def _st_tile(name):
                # startup only: borrow the (idle) score pool's banks so the
                # first projection chunks pipeline 4-deep instead of 2-deep
                return stp.tile([128, 2, 512], F32, tag="st", name=name)[:, 0, :]

            def c_qk(m, w_sb, dst, mk=_pp_tile):
                def f():
                    xts = xts_cell[m]
                    sl = slice(512 * m, 512 * (m + 1))
                    pj = mk("pj")
                    for ko in range(KO):
                        nc.tensor.matmul(pj[:], w_sb[:, ko], xts[:, ko],
                                         start=(ko == 0), stop=(ko == KO - 1))
                    r1 = work.tile([128, 512], BF16, tag="r1", name="r1")
                    r2 = work.tile([128, 512], BF16, tag="r2", name="r2")
                    r2s = work.tile([128, 512], BF16, tag="r2s", name="r2s")
                    nc.vector.tensor_mul(r1[:], pj[:], cc_sb[:, sl])
                    nc.vector.tensor_mul(r2[:], pj[:], ss_sb[:, sl])
                    nc.vector.stream_shuffle(r2s[:], r2[:], SWAP_MASK)
                    nc.vector.tensor_add(dst[m][:], r1[:], r2s[:])
                return f

            def c_v(m, half, mk=_pp_tile):
                def f():
                    xts = xts_cell[m]
                    pv = mk("pv")
                    for sc in range(2):
                        xsl = slice(128 * (2 * half + sc),
                                    128 * (2 * half + sc) + 128)
                        for ko in range(KO):
                            nc.tensor.matmul(
                                pv[:, 256 * sc:256 * sc + 256],
                                xts[:, ko, xsl], wv_sb[:, ko],
                                start=(ko == 0), stop=(ko == KO - 1))
                    nc.vector.tensor_copy(
                        vt[m][:, 2 * half:2 * half + 2, :, 0:DH],
                        pv[:].rearrange("p (sc h d) -> p sc h d", sc=2, d=DH))
                return f

            def proj_chunks(m):
                return [c_dma(m),
                        c_qk(m, wqa_sb, qr[0]), c_qk(m, wka_sb, kr[0]),
                        c_v(m, 0), c_v(m, 1),
                        c_qk(m, wqb_sb, qr[1]), c_qk(m, wkb_sb, kr[1])]

            # slice 0 (xts0 already DMA'd above): emit what macro-0 p=0
            # needs up front, pipelined through the still-idle score pool;
            # QB/KB interleave into the p=0 stream
            c_qk(0, wqa_sb, qr[0])()
            c_qk(0, wka_sb, kr[0])()
            c_v(0, 0)()
            c_v(0, 1)()
            pending = [c_dma_w1] + proj_chunks(0)[5:]
            need_before_p1 = len(pending)   # wqb/wkb+QB0/KB0 precede p=1

            # ---------------- attention ----------------
            def normalize(p, m, at):
                def f():
                    for a in range(2):
                        ssb = work.tile([1, 512], BF16, tag="ssb", name="ssb")
                        rbc = work.tile([64, 512], F32, tag="rbc", name="rbc")
                        nc.vector.tensor_copy(ssb[:], at[a][DH:DH + 1, :])
                        sbc = pp.tile([128, 512], F32, tag="pp", name="sbc")
                        nc.tensor.matmul(sbc[0:64, :], ones1, ssb[:],
                                         start=True, stop=True)
                        nc.vector.reciprocal_approx_fast(rbc[:], sbc[0:64, :])
                        nc.vector.tensor_mul(
                            atn[p][m][64 * a:64 * a + 64, :], at[a][0:DH, :],
                            rbc[:])
                return f

            def outproj_chunk(m, sc):
                def f():
                    scl = slice(128 * (sc % 4), 128 * (sc % 4) + 128)
                    osb = work.tile([128, D], BF16, tag="osb", name="osb")
                    po = [pp.tile([128, 512], F32, tag="pp", name=f"po{nh}")
                          for nh in range(2)]
                    for nh in range(2):
                        for ksub in range(2):
                            nc.tensor.matmul(
                                po[nh][:], atn[ksub][m][:, scl],
                                wo_sb[:, ksub, 512 * nh:512 * nh + 512],
                                start=(ksub == 0), stop=(ksub == 1))
                    for nh in range(2):
                        nc.vector.tensor_copy(
                            osb[:, 512 * nh:512 * nh + 512], po[nh][:])
                    nc.sync.dma_start(o_part[128 * sc:128 * sc + 128, :], osb[:])
                return f

            deferred = [None]   # normalize of the previous (m, p) group

            for m in range(N_SLICES):
                if m > 0:
                    pending = [outproj_chunk(m - 1, sc)
                               for sc in range(4 * (m - 1), 4 * (m - 1) + 4)]
                    if m == 1:
                        pending.insert(0, c_dma_w2)
                    need_before_p1 = 0
                if m + 1 < N_SLICES:
                    pending.extend(proj_chunks(m + 1))
                nkb = 4 * m + 4
                iters = 2 * nkb
                it = 0
                emitted = 0
                for p in range(2):
                    sts = {}

                    def emit_scores(kb, p=p, m=m, sts=sts):
                        st = stp.tile([128, 2, 512], F32, tag="st", name="st")
                        sts[kb] = st
                        ksl = slice(128 * (kb % 4), 128 * (kb % 4) + 128)
                        c0 = 128 * (kb % 4) if (kb // 4 == m) else 0
                        for a in range(2):
                            nc.tensor.matmul(
                                st[:, a, c0:],
                                kr[p][kb // 4][64 * a:64 * a + 64, ksl],
                                qr[p][m][64 * a:64 * a + 64, c0:],
                                start=True, stop=True)

                    if p == 1:
                        while emitted < need_before_p1:
                            pending[emitted]()
                            emitted += 1
                    emit_scores(0)
                    # flush the previous group's normalize now, before the
                    # at-pool slots are reallocated below (WAR ordering)
                    if deferred[0] is not None:
                        deferred[0]()
                        deferred[0] = None
                    at = [atp.tile([DH + 1, 512], F32, tag=f"at{a}",
                                   name=f"at{a}") for a in range(2)]
                    for kb in range(nkb):
                        diag = (kb // 4 == m)
                        c0 = 128 * (kb % 4) if diag else 0
                        st = sts.pop(kb)
                        pt = work.tile([128, 2, 512], BF16, tag="pt", name="pt",
                                       bufs=4)
                        nc.scalar.activation(
                            pt[:, :, c0:], st[:, :, c0:], Exp, scale=EXP_SCALE)
                        if diag:
                            w = 512 - c0
                            nc.gpsimd.affine_select(
                                pt[:, :, c0:], pt[:, :, c0:],
                                pattern=[[0, 2], [1, w]],
                                compare_op=GE, fill=0.0,
                                base=0, channel_multiplier=-1)
                        if kb + 1 < nkb:
                            emit_scores(kb + 1)
                        for a in range(2):
                            nc.tensor.matmul(
                                at[a][:, c0:], vt[kb // 4][:, kb % 4, 2 * p + a],
                                pt[:, a, c0:],
                                start=(kb == 0), stop=(kb == nkb - 1))
                        # interleave pending chunks (outproj m-1, proj m+1)
                        it += 1
                        want = (it * len(pending)) // max(iters, 1)
                        while emitted < want and emitted < len(pending):
                            pending[emitted]()
                            emitted += 1

                    deferred[0] = normalize(p, m, at)
                while emitted < len(pending):
                    pending[emitted]()
                    emitted += 1
                pending = []

            # tail: last normalize + last macro's output projection
            deferred[0]()
            for sc in range(4 * (N_SLICES - 1), 4 * N_SLICES):
                outproj_chunk(N_SLICES - 1, sc)()

    nc.finalize()
    return nc


def _to_bf16(x):
    from ml_dtypes import bfloat16
    return np.asarray(x, dtype=np.float32).astype(bfloat16)


def prep_inputs(hidden_states, cos, sin, attention_mask, Wq, Wk, Wv, Wo):
    """Host-side sharding/layout prep. Returns in_maps for the 8 cores."""
    hs = np.asarray(hidden_states, dtype=np.float32)
    cos = np.asarray(cos, dtype=np.float32)
    sin = np.asarray(sin, dtype=np.float32)
    Wq = np.asarray(Wq, dtype=np.float32)
    Wk = np.asarray(Wk, dtype=np.float32)
    Wv = np.asarray(Wv, dtype=np.float32)
    Wo = np.asarray(Wo, dtype=np.float32)

    # rope tables in pair-interleaved layout, 2 heads (128 partitions) per tile
    idx = np.empty(64, dtype=np.int64)
    idx[0::2] = np.arange(32)
    idx[1::2] = np.arange(32)
    cc1 = cos.T[idx]                      # [64, S]
    ss1 = sin.T[idx].copy()               # [64, S]
    ss1[1::2] *= -1.0
    ccT = np.ascontiguousarray(np.tile(cc1, (2, 1)), dtype=np.float32)  # [128,S]
    ssT = np.ascontiguousarray(np.tile(ss1, (2, 1)), dtype=np.float32)

    # per-head column permutation of Wq/Wk into interleaved order
    perm = np.empty(64, dtype=np.int64)
    perm[0::2] = np.arange(32)        # x1_i = dim i
    perm[1::2] = np.arange(32) + 32   # x2_i = dim 32+i

    xTs = [np.ascontiguousarray(_to_bf16(hs[b].T)) for b in range(B)]

    def _prearr(w):
        # [ko*128, m] -> [128, ko*m] so per-partition DMA runs are contiguous
        ko, m = w.shape[0] // 128, w.shape[1]
        return np.ascontiguousarray(_to_bf16(
            w.reshape(ko, 128, m).transpose(1, 0, 2).reshape(128, ko * m)))

    in_maps = []
    for c in range(N_CORES):
        b, g = c // 4, c % 4
        hsl = slice(DG * g, DG * (g + 1))
        wq_g = Wq[:, hsl].reshape(D, HPC, DH)[:, :, perm]   # [D, 4, 64]
        wk_g = Wk[:, hsl].reshape(D, HPC, DH)[:, :, perm]
        in_maps.append({
            "xT": xTs[b],
            "wqa": _prearr(wq_g[:, 0:2].reshape(D, 128)),
            "wqb": _prearr(wq_g[:, 2:4].reshape(D, 128)),
            "wka": _prearr(wk_g[:, 0:2].reshape(D, 128)),
            "wkb": _prearr(wk_g[:, 2:4].reshape(D, 128)),
            "wv": _prearr(Wv[:, hsl]),
            "wo": _prearr(Wo[hsl, :]),
            "ccT": ccT,
            "ssT": ssT,
        })
    return in_maps


_NC_CACHE = {}


def get_nc():
    if "nc" not in _NC_CACHE:
        _NC_CACHE["nc"] = build_nc()
    return _NC_CACHE["nc"]


def run(inputs, trace=False):
    """Returns (output [B,S,D] fp32, BassKernelResults)."""
    nc = get_nc()
    in_maps = prep_inputs(**inputs)
    res = run_bass_kernel_spmd(nc, in_maps, list(range(N_CORES)), trace=trace)
    out = np.zeros((B, S, D), dtype=np.float32)
    for c in range(N_CORES):
        out[c // 4] += np.asarray(res.results[c]["o_part"], dtype=np.float32)
    return out, res


def kernel(**inputs):
    return run(inputs, trace=False)[0]
